# revision 27
# baseline (speedup 1.0000x reference)
"""Fused QKV projection (nn.Linear premix) on 8 Trainium2 NeuronCores.

qkv = x @ W_qkv^T ; split into per-head q,k,v of shape [B,H,S,DK].

Sharding (tensor-parallel, per spec hint): the 3E=6144 output dim of
W_qkv is head-sharded across 8 cores.  Core c owns q-heads {2c,2c+1},
k-heads {2c,2c+1}, v-heads {2c,2c+1} -> 768 rows of W.  x is replicated.

Per-core GEMM: [16384 x 2048] @ [2048 x 768] in bf16 (TensorE peak
78.6 TF/s -> ~656 us floor; measured stream rate 216 ns per N=512 MM).

Device kernel design (v9):
  - W-stationary matmuls: lhsT = W^T block [128k, 128f], moving rhs =
    x^T [128k, 512 tokens] -> PSUM [128f, 512tok] fp32 = exactly one
    full PSUM bank.  3072 MMs total; 216 ns each, LDWEIGHTS fully
    hidden.
  - Host pre-layouts so every DMA line is long and contiguous:
      xh  [32, 128, 16, 512] : xh[st,p,kt,m] = x[st*512+m, kt*128+p]
          -> one 2 MB DMA per 512-token super-tile with 16 KB lines
             (supertile 0 split into 8x256 KB chunks).
      wt  [128, 16, 768]     : wt[p,kt,f] = W_c[f, kt*128+p]
          -> 8 chunks of 2 kt (384 KB, 3 KB lines).
  - Warm-up burst (~30 N=128 matmuls on a zeroed tile) keeps the PE
    busy from right after the preamble barrier so the HAM clock gate is
    at 8/8 (2.4 GHz) before the first real matmul.
  - Head is data-bound on (W 3MB + x-st0 2MB): supertile 0 runs all
    SIX fb-chains INTERLEAVED, sweeping kt-pairs, with 6 PSUM banks
    open at once.  Each (W-chunk + x0-chunk) pair (640 KB) unlocks
    12 matmuls (2.6 us) across the chains, so the PE consumes the head
    supply incrementally and never idles long enough to re-throttle.
  - Steady state (st 1..31): per fb chain of 16 matmuls, VectorE casts
    PSUM fp32 -> bf16, one 128 KB DMA per chain on the scalar ring
    writes out[fb, token-slice].  The final chain is split in two
    N=256 halves so the last PSUM drain + out-DMA receipt is shorter.
  - Queue split: all inputs on the sync HWDGE ring, outputs on the
    scalar ring.
"""

import numpy as np
import ml_dtypes

B, S, E, H, DK = 4, 4096, 2048, 16, 128
M = B * S              # 16384 tokens
NCORES = 8
FPC = 3 * E // NCORES  # 768 output features per core (6 head-slices)
FB = FPC // 128        # 6 feature blocks (head-slices)
KT = E // 128          # 16 contraction subtiles
TOK_SUPER = 512
N_SUPER = M // TOK_SUPER

_cache = {}


def _build_program():
    import concourse.bass as bass
    import concourse.bacc as bacc
    import concourse.mybir as mybir
    from concourse import tile

    ts = bass.ts
    nc = bacc.Bacc("TRN2", target_bir_lowering=False, debug=False,
                   num_devices=NCORES)
    xh = nc.dram_tensor("xh", [N_SUPER, 128, KT, TOK_SUPER],
                        mybir.dt.bfloat16, kind="ExternalInput")
    wt = nc.dram_tensor("wt", [128, KT, FPC], mybir.dt.bfloat16,
                        kind="ExternalInput")
    out = nc.dram_tensor("out", [FPC, M], mybir.dt.bfloat16,
                         kind="ExternalOutput")

    KC = 8                 # kt chunks for W and for supertile 0's x
    KPC = KT // KC         # 2 kt per chunk
    with tile.TileContext(nc) as tc:
        with tc.tile_pool(name="zpool", bufs=1) as zpool, \
             tc.tile_pool(name="wpool", bufs=1) as wpool, \
             tc.tile_pool(name="x0pool", bufs=1) as x0pool, \
             tc.tile_pool(name="xpool", bufs=3) as xpool, \
             tc.tile_pool(name="opool", bufs=4) as opool, \
             tc.tile_pool(name="wmps", bufs=1, space="PSUM") as wmps, \
             tc.tile_pool(name="pspool", bufs=6, space="PSUM") as pspool:
            # ---- warm-up: PE busy from ~t0 so HAM un-throttles before
            # the first real matmul; zero source, dedicated PSUM bank.
            zt = zpool.tile([128, 512], mybir.dt.bfloat16, tag="z")
            nc.vector.memset(zt[:], 0.0)
            pw = wmps.tile([128, 512], mybir.dt.float32, tag="pw")
            for _ in range(30):
                nc.tensor.matmul(pw[:, 0:128], zt[:, 0:128], zt[:, 0:128],
                                 start=True, stop=True)

            # ---- W and x-supertile-0 chunks pairwise interleaved on
            # the sync ring: each (W, x0) pair unlocks one kt-sweep
            # across the six interleaved st0 chains.  kt0 and kt1 ride
            # as 1-kt pieces (320 KB gate) so the first matmul starts
            # as early as possible; the rest are 2-kt chunks.
            wsb = []    # (tile, kt0, nkt)
            x0 = []
            pieces = [(0, 1), (1, 1)] + [(k, 2) for k in range(2, KT, 2)]
            for kt0, nkt in pieces:
                wc = wpool.tile([128, nkt, FPC], mybir.dt.bfloat16,
                                tag=f"w{kt0}")
                nc.sync.dma_start(wc[:], wt[:, kt0:kt0 + nkt, :])
                wsb.append((wc, kt0, nkt))
                xc = x0pool.tile([128, nkt, TOK_SUPER], mybir.dt.bfloat16,
                                 tag=f"x0{kt0}")
                nc.sync.dma_start(xc[:], xh[0, :, kt0:kt0 + nkt, :])
                x0.append((xc, kt0, nkt))

            def piece_idx(kt):
                return kt if kt < 2 else 2 + (kt - 2) // 2

            def wslice(fb, kt):
                wc, kt0, _ = wsb[piece_idx(kt)]
                return wc[:, kt - kt0, ts(fb, 128)]

            # ---- supertile 0: six interleaved chains, kt-major sweep.
            ps0 = [pspool.tile([128, TOK_SUPER], mybir.dt.float32,
                               name=f"ps0_{fb}", tag="ps")
                   for fb in range(FB)]
            for kt in range(KT):
                xc, kt0, _ = x0[piece_idx(kt)]
                xv = xc[:, kt - kt0, :]
                for fb in range(FB):
                    nc.tensor.matmul(ps0[fb][:], wslice(fb, kt), xv,
                                     start=(kt == 0), stop=(kt == KT - 1))
            for fb in range(FB):
                osb = opool.tile([128, TOK_SUPER], mybir.dt.bfloat16)
                nc.vector.tensor_copy(osb[:], ps0[fb][:])
                nc.scalar.dma_start(out[ts(fb, 128), ts(0, TOK_SUPER)],
                                    osb[:])

            # ---- supertiles 1..31: sequential chains (fb-major)
            for st in range(1, N_SUPER):
                xs = xpool.tile([128, KT, TOK_SUPER], mybir.dt.bfloat16,
                                tag="xs")
                nc.sync.dma_start(xs[:], xh[st])
                for fb in range(FB):
                    if st == N_SUPER - 1 and fb == FB - 1:
                        # final chain: two N=256 halves so the last
                        # PSUM drain + out-DMA receipt is half-size
                        for h in range(2):
                            ps = pspool.tile([128, TOK_SUPER],
                                             mybir.dt.float32, tag="ps")
                            for kt in range(KT):
                                nc.tensor.matmul(
                                    ps[:, 0:TOK_SUPER // 2], wslice(fb, kt),
                                    xs[:, kt, ts(h, TOK_SUPER // 2)],
                                    start=(kt == 0), stop=(kt == KT - 1))
                            osb = opool.tile([128, TOK_SUPER // 2],
                                             mybir.dt.bfloat16)
                            nc.vector.tensor_copy(osb[:], ps[:, 0:TOK_SUPER // 2])
                            nc.scalar.dma_start(
                                out[ts(fb, 128),
                                    st * TOK_SUPER + h * (TOK_SUPER // 2):
                                    st * TOK_SUPER + (h + 1) * (TOK_SUPER // 2)],
                                osb[:])
                        continue
                    ps = pspool.tile([128, TOK_SUPER], mybir.dt.float32,
                                     tag="ps")
                    for kt in range(KT):
                        nc.tensor.matmul(ps[:], wslice(fb, kt),
                                         xs[:, kt, :],
                                         start=(kt == 0), stop=(kt == KT - 1))
                    osb = opool.tile([128, TOK_SUPER], mybir.dt.bfloat16)
                    nc.vector.tensor_copy(osb[:], ps[:])
                    nc.scalar.dma_start(
                        out[ts(fb, 128), ts(st, TOK_SUPER)], osb[:])
    nc.compile()
    return nc


def _host_inputs(x, W_qkv):
    bf16 = ml_dtypes.bfloat16
    xf = np.asarray(x, dtype=np.float32).reshape(M, E).astype(bf16)
    # xh[st, p, kt, m] = x[st*512+m, kt*128+p]
    xh = np.ascontiguousarray(
        xf.reshape(N_SUPER, TOK_SUPER, KT, 128).transpose(0, 3, 2, 1))
    W = np.asarray(W_qkv, dtype=np.float32)
    in_maps = []
    for c in range(NCORES):
        rows = np.concatenate([W[o + 256 * c: o + 256 * c + 256]
                               for o in (0, E, 2 * E)])
        # wt[p, kt, f] = W_c[f, kt*128+p]
        wt_c = np.ascontiguousarray(
            rows.reshape(FPC, KT, 128).astype(bf16).transpose(2, 1, 0))
        in_maps.append({"xh": xh, "wt": wt_c})
    return in_maps


def kernel(x, W_qkv):
    from concourse.bass_utils import run_bass_kernel_spmd

    if "nc" not in _cache:
        _cache["nc"] = _build_program()
    nc = _cache["nc"]

    in_maps = _host_inputs(x, W_qkv)
    res = run_bass_kernel_spmd(nc, in_maps, core_ids=list(range(NCORES)))
    kernel._last_results = res

    q = np.empty((B, H, S, DK), np.float32)
    k = np.empty_like(q)
    v = np.empty_like(q)
    for c in range(NCORES):
        o = res.results[c]["out"]                       # [768, 16384] bf16
        # arr[b, fb, s, dk] = o[fb*128+dk, b*4096+s]
        arr = np.ascontiguousarray(
            o.reshape(FB, 128, B, S).transpose(2, 0, 3, 1)).astype(np.float32)
        for j in range(2):
            q[:, 2 * c + j] = arr[:, j]
            k[:, 2 * c + j] = arr[:, 2 + j]
            v[:, 2 * c + j] = arr[:, 4 + j]
    return q, k, v
